# revision 1
# baseline (speedup 1.0000x reference)
"""ChebNet (K=3) on 8 Trainium2 NeuronCores — v2.

Key changes vs baseline:
- All gather tables, staircases and segsum matmuls in bf16 (PE 4x, half DMA).
- dma_gather round-robins SWDGE queues 0-3 (2 Q7 cores per queue; Pool
  pipeline depth 2 gives ~2x descriptor-gen throughput).
- Parity-split chunks (layer 0): one matmul + 128-wide staircase per chunk
  instead of two slots (halves DVE staircase + PE work).
- Per-tile-position exact index counts; padding idx = -1 (ucode trims
  trailing negatives, dummy-descs the rest).
- x1/x2/xp/feat copies SBUF-resident (no DRAM round trip for dense layers).
- AllGathers split in halves, overlapped with producer tiles; gather tables
  use an AG-half-major layout with host-side index remap.
"""

import numpy as np
import ml_dtypes

import concourse.bass as bass  # noqa: F401
import concourse.bacc as bacc
import concourse.mybir as mybir
import concourse.tile as tile
import concourse.bass_utils as bass_utils
from concourse.library_config import mlp

F32 = mybir.dt.float32
BF16 = mybir.dt.bfloat16
I16 = mybir.dt.int16
AX = mybir.AxisListType
OP = mybir.AluOpType
AF = mybir.ActivationFunctionType
BF = ml_dtypes.bfloat16

P = 128
NQ = 4          # SWDGE queues round-robinned across gathers
NEGPAD = False  # -1 idx padding crashes the gather ucode path; keep 0-pad
SHARED = True
SPLITAG = True  # split AllGathers in halves, overlapped with producers

# --------------------------------------------------------------------------
# host-side graph preprocessing
# --------------------------------------------------------------------------


def _wrap_idx(vals):
    """[n] -> [128, n//16] dma_gather idx layout (16-wrap, replicated x8)."""
    n = len(vals)
    blk = vals.reshape(n // 16, 16).T
    return np.tile(blk, (8, 1))


AGK = 2  # AllGather split factor


def _remap_rows(rows, SH, NC):
    """Map table row -> AG-part-major layout row (AGK AllGather parts)."""
    c = rows // SH
    loc = rows - c * SH
    part = SH // AGK
    h = loc // part
    return h * (NC * part) + c * part + (loc - h * part)


def _plan_layer(src, dst, N, NC, pair, rowN, rowSH):
    """Build per-core idx/dl tables + static per-tile-position chunk plan.

    pair: indices are src//2 (parity-split chunks); else src directly.
    rowN/rowSH: gather-table row count and per-core row shard (for remap).

    Returns (plan, per_core) where plan has static lists ch[t], qlist[t],
    nv16[t] (num_idxs_reg), offsets; per_core[c] = {idx, dl} arrays.
    """
    SH = N // NC
    T = SH // P
    NT = N // P
    order = np.argsort(dst, kind="stable")
    ds = dst[order]
    ss = src[order].astype(np.int64)
    bounds = np.searchsorted(ds, np.arange(0, N + 1, P))

    # per (core, pos): even/odd edge lists (or single list)
    ecnt = np.zeros((NC, T), np.int64)
    ocnt = np.zeros((NC, T), np.int64)
    edges = {}
    for g in range(NT):
        c, t = g // T, g % T
        sl = ss[bounds[g]:bounds[g + 1]]
        dl = (ds[bounds[g]:bounds[g + 1]] - g * P).astype(np.int64)
        if pair:
            par = (sl % 2).astype(np.int64)
            ev, od = par == 0, par == 1
            edges[(c, t)] = ((sl[ev] // 2, dl[ev]), (sl[od] // 2, dl[od]))
            ecnt[c, t], ocnt[c, t] = ev.sum(), od.sum()
        else:
            edges[(c, t)] = ((sl, dl), (np.zeros(0, np.int64),) * 2)
            ecnt[c, t] = len(sl)

    ech = np.maximum(1, -(-ecnt.max(axis=0) // P))          # [T]
    och = -(-ocnt.max(axis=0) // P) if pair else np.zeros(T, np.int64)
    ch = (ech + och).astype(np.int64)
    # num_idxs_reg: cover last possibly-valid slot, rounded to 16
    if pair:
        nv = ech * P + ocnt.max(axis=0)
        nv = np.where(ocnt.max(axis=0) > 0, nv, ecnt.max(axis=0))
    else:
        nv = ecnt.max(axis=0)
    nv16 = np.minimum(-(-nv // 16) * 16, ch * P).astype(np.int64)
    nv16 = np.maximum(nv16, 16)
    if not NEGPAD:
        nv16 = (ch * P).astype(np.int64)

    qlist = [[0] * int(ech[t]) + [1] * int(och[t]) for t in range(T)]
    off = np.concatenate([[0], np.cumsum(ch)]).astype(np.int64)  # chunk prefix

    per_core = []
    for c in range(NC):
        idx_cols = []
        dl_cols = []
        for t in range(T):
            (se, de), (so, do) = edges[(c, t)]
            il = np.full(int(ch[t]) * P, -1 if NEGPAD else 0, np.int64)
            dv = np.full(int(ch[t]) * P, -1.0, np.float32)
            il[: len(se)] = se
            dv[: len(se)] = de
            ob = int(ech[t]) * P
            il[ob: ob + len(so)] = so
            dv[ob: ob + len(so)] = do
            valid = np.zeros(len(il), bool); valid[:len(se)] = True
            valid[ob: ob + len(so)] = True
            il[valid] = _remap_rows(il[valid], rowSH, NC)
            assert il.max() < 32768
            idx_cols.append(_wrap_idx(il.astype(np.int16)))
            dl_cols.append(dv.reshape(int(ch[t]), P).T)  # [128, ch]
        per_core.append({
            "idx": np.concatenate(idx_cols, axis=1).astype(np.int16),
            "dl": np.concatenate(dl_cols, axis=1).astype(BF),
        })
    plan = {
        "T": T, "ch": [int(x) for x in ch], "q": qlist,
        "nv16": [int(x) for x in nv16],
        "off": [int(x) for x in off],
        "chmax": int(ch.max()),
        "totch": int(ch.sum()),
    }
    return plan, per_core


def _dinv_cols(dinv, base, SH):
    return dinv[base:base + SH].reshape(SH // P, P).T.copy()


# --------------------------------------------------------------------------
# device program
# --------------------------------------------------------------------------

def _emit_prop(nc, sb, ps, plan, tabs, gstate, F, finalize):
    """One propagation pass over this core's dst shard."""
    T, ch, q, nv16, off = (plan["T"], plan["ch"], plan["q"], plan["nv16"],
                           plan["off"])
    table, idx_t, dl_t = tabs
    gbufs = gstate["gbufs"]
    iota = gstate["iota"]
    for t in range(T):
        c = ch[t]
        gb = gbufs[gstate["gi"] % len(gbufs)]
        gstate["gi"] += 1
        nc.gpsimd.dma_gather(
            out_ap=gb[:, : c * 128].rearrange("p (g f) -> p g f", g=c),
            in_ap=table[:],
            idxs_ap=idx_t[:, off[t] * 8: (off[t] + c) * 8],
            num_idxs=c * P,
            num_idxs_reg=nv16[t],
            elem_size=128,
            single_packet=False,
            queue_num=gstate["gi"] % NQ,
        )
        s_all = gstate["sbs"].tile([P, plan["chmax"] * P], BF16, tag="s_all",
                                   name=f"sa{gstate['gi'] % 4}")
        nc.vector.tensor_tensor(
            out=s_all[:, : c * P].rearrange("p (c j) -> p c j", c=c),
            in0=dl_t[:, off[t]: off[t] + c].to_broadcast([P, c, P]),
            in1=iota[:].rearrange("p (c j) -> p c j", c=1).to_broadcast([P, c, P]),
            op=OP.is_equal,
        )
        seg = ps.tile([P, F], F32, tag="seg")
        for k in range(c):
            rhs = (gb[:, k * 128 + q[t][k] * F: k * 128 + (q[t][k] + 1) * F]
                   if F != 128 else gb[:, k * 128: (k + 1) * 128])
            nc.tensor.matmul(
                out=seg[:],
                lhsT=s_all[:, k * P: (k + 1) * P],
                rhs=rhs,
                start=(k == 0),
                stop=(k == c - 1),
            )
        finalize(t, seg)


def build_program(cfg):
    N0, N1, IN, H, OUT, NC = (cfg["N0"], cfg["N1"], cfg["IN"], cfg["H"],
                              cfg["OUT"], cfg["NC"])
    SH0, SH1 = N0 // NC, N1 // NC
    T0, T1, TP = SH0 // P, SH1 // P, SH0 // 2 // P
    pl0, pl1 = cfg["plan0"], cfg["plan1"]
    PR = N0 // 2  # pair rows
    PRSH = PR // NC

    nc = bacc.Bacc("TRN2", target_bir_lowering=False, debug=False,
                   num_devices=NC, num_swdge_queues=NQ)

    # ---- I/O ----
    feat_sh = nc.dram_tensor("feat_sh", [SH0, IN], F32, kind="ExternalInput").ap()
    idx0_d = nc.dram_tensor("idx0", [P, pl0["totch"] * 8], I16, kind="ExternalInput").ap()
    dl0_d = nc.dram_tensor("dl0", [P, pl0["totch"]], BF16, kind="ExternalInput").ap()
    idx1_d = nc.dram_tensor("idx1", [P, pl1["totch"] * 8], I16, kind="ExternalInput").ap()
    dl1_d = nc.dram_tensor("dl1", [P, pl1["totch"]], BF16, kind="ExternalInput").ap()
    pdv0_d = nc.dram_tensor("pdv0", [P, T0], F32, kind="ExternalInput").ap()
    ndv0_d = nc.dram_tensor("ndv0", [P, T0], F32, kind="ExternalInput").ap()
    nd2v0_d = nc.dram_tensor("nd2v0", [P, T0], F32, kind="ExternalInput").ap()
    n2dv0_d = nc.dram_tensor("n2dv0", [P, T0], F32, kind="ExternalInput").ap()
    pdv1s_d = nc.dram_tensor("pdv1s", [P, TP], F32, kind="ExternalInput").ap()
    ndv1_d = nc.dram_tensor("ndv1", [P, T1], F32, kind="ExternalInput").ap()
    nd2v1_d = nc.dram_tensor("nd2v1", [P, T1], F32, kind="ExternalInput").ap()
    n2dv1_d = nc.dram_tensor("n2dv1", [P, T1], F32, kind="ExternalInput").ap()
    w0_d = nc.dram_tensor("W0", [3 * IN, H], F32, kind="ExternalInput").ap()
    b0_d = nc.dram_tensor("b0r", [P, H], F32, kind="ExternalInput").ap()
    w1_d = nc.dram_tensor("W1", [3 * H, H], F32, kind="ExternalInput").ap()
    b1_d = nc.dram_tensor("b1r", [P, H], F32, kind="ExternalInput").ap()
    wc_d = nc.dram_tensor("Wc", [H, OUT], F32, kind="ExternalInput").ap()
    bc_d = nc.dram_tensor("bcr", [1, OUT], F32, kind="ExternalInput").ap()
    iota_d = nc.dram_tensor("iotaB", [P, P], BF16, kind="ExternalInput").ap()
    ident_d = nc.dram_tensor("ident", [P, P], F32, kind="ExternalInput").ap()
    y_d = nc.dram_tensor("y", [1, OUT], F32, kind="ExternalOutput").ap()

    # ---- internal DRAM ----
    xd0_sh = nc.dram_tensor("xd0_sh", [PRSH, 2 * IN], BF16).ap()
    xd0_full = nc.dram_tensor("xd0_full", [PR, 2 * IN], BF16,
                              addr_space=("Shared" if SHARED else "Local")).ap()
    x1d_sh = nc.dram_tensor("x1d_sh", [PRSH, 2 * IN], BF16).ap()
    x1d_full = nc.dram_tensor("x1d_full", [PR, 2 * IN], BF16,
                              addr_space=("Shared" if SHARED else "Local")).ap()
    hbuf = nc.dram_tensor("hbuf", [SH0, H], BF16).ap()
    xpd_sh = nc.dram_tensor("xpd_sh", [SH1, H], BF16).ap()
    xpd_full = nc.dram_tensor("xpd_full", [N1, H], BF16,
                              addr_space=("Shared" if SHARED else "Local")).ap()
    x1d1_sh = nc.dram_tensor("x1d1_sh", [SH1, H], BF16).ap()
    x1d1_full = nc.dram_tensor("x1d1_full", [N1, H], BF16,
                               addr_space=("Shared" if SHARED else "Local")).ap()
    gmax_in = nc.dram_tensor("gmax_in", [P, 1], F32).ap()
    gmax_out = nc.dram_tensor("gmax_out", [P, 1], F32).ap()

    groups = [list(range(NC))]

    _AGK = AGK

    def ag_part(src_ap, dst_ap, rows_sh, h):
        part = rows_sh // _AGK
        nc.gpsimd.collective_compute(
            "AllGather", OP.bypass, replica_groups=groups,
            ins=[src_ap[h * part:(h + 1) * part, :].opt()],
            outs=[dst_ap[h * part * NC:(h + 1) * part * NC, :].opt()])

    with tile.TileContext(nc) as tc:
        nc.gpsimd.load_library(mlp)
        with (
            tc.tile_pool(name="sb", bufs=2) as sb,
            tc.tile_pool(name="sb1", bufs=1) as sb1,
            tc.tile_pool(name="sbs", bufs=4) as sbs,
            tc.tile_pool(name="ps", bufs=2, space="PSUM") as ps,
        ):
            # ---- consts / weights ----
            iota = sb1.tile([P, P], BF16, name="iota")
            nc.sync.dma_start(iota[:], iota_d[:])
            ident = sb1.tile([P, P], F32, name="ident")
            nc.sync.dma_start(ident[:], ident_d[:])
            w0a = sb1.tile([P, H], F32, name="w0a")
            nc.sync.dma_start(w0a[:], w0_d[:P, :])
            w0b = sb1.tile([P, H], F32, name="w0b")
            nc.sync.dma_start(w0b[: 3 * IN - P, :], w0_d[P:, :])
            w1_sb = sb1.tile([P, 3 * H], F32, name="w1sb")
            for i in range(3):
                nc.sync.dma_start(w1_sb[:, i * H:(i + 1) * H],
                                  w1_d[i * P:(i + 1) * P, :])
            wc_sb = sb1.tile([P, OUT], F32, name="wcsb")
            nc.sync.dma_start(wc_sb[:], wc_d[:])
            b0_sb = sb1.tile([P, H], F32, name="b0sb")
            nc.sync.dma_start(b0_sb[:], b0_d[:])
            b1_sb = sb1.tile([P, H], F32, name="b1sb")
            nc.sync.dma_start(b1_sb[:], b1_d[:])
            bc_sb = sb1.tile([1, OUT], F32, name="bcsb")
            nc.sync.dma_start(bc_sb[:], bc_d[:])
            dvt = {}
            for nm, ap_, w in [("pdv0", pdv0_d, T0), ("ndv0", ndv0_d, T0),
                               ("nd2v0", nd2v0_d, T0), ("n2dv0", n2dv0_d, T0),
                               ("pdv1s", pdv1s_d, TP), ("ndv1", ndv1_d, T1),
                               ("nd2v1", nd2v1_d, T1), ("n2dv1", n2dv1_d, T1)]:
                tl = sb1.tile([P, w], F32, name=nm + "sb")
                nc.sync.dma_start(tl[:], ap_[:])
                dvt[nm] = tl

            # ---- idx / dl tables ----
            idx0_t = sb1.tile([P, pl0["totch"] * 8], I16, name="idx0t")
            nc.sync.dma_start(idx0_t[:], idx0_d[:])
            dl0_t = sb1.tile([P, pl0["totch"]], BF16, name="dl0t")
            nc.sync.dma_start(dl0_t[:], dl0_d[:])
            idx1_t = sb1.tile([P, pl1["totch"] * 8], I16, name="idx1t")
            nc.sync.dma_start(idx1_t[:], idx1_d[:])
            dl1_t = sb1.tile([P, pl1["totch"]], BF16, name="dl1t")
            nc.sync.dma_start(dl1_t[:], dl1_d[:])

            # ---- persistent SBUF feature stores (bf16) ----
            x0_sb = sb1.tile([P, T0 * IN], BF16, name="x0sb")
            x1_sb = sb1.tile([P, T0 * IN], BF16, name="x1sb")
            x2_sb = sb1.tile([P, T0 * IN], BF16, name="x2sb")
            xp_sb = sb1.tile([P, T1 * H], BF16, name="xpsb")
            x11_sb = sb1.tile([P, T1 * H], BF16, name="x11sb")
            x21_sb = sb1.tile([P, T1 * H], BF16, name="x21sb")

            # ---- gather buffers ----
            GW = max(pl0["chmax"], pl1["chmax"]) * 128
            gbufs = []
            for i in range(8):
                g = sb1.tile([P, GW], BF16, name=f"gbuf{i}")
                nc.vector.memset(g[:], 0.0)
                gbufs.append(g)
            gstate = {"gbufs": gbufs, "gi": 0, "iota": iota, "sbs": sbs}

            # ================= scale pass =================
            NB = 8
            for t0 in range(0, T0, NB):
                ft = sb.tile([P, NB * IN], F32, tag="scl", name="sclf")
                nc.sync.dma_start(
                    ft[:].rearrange("p (b f) -> p b f", b=NB),
                    feat_sh[:].rearrange("(b p) f -> p b f", p=P)[:, t0:t0 + NB, :])
                xo = sb.tile([P, NB * IN], BF16, tag="sclo", name="sclo")
                nc.vector.tensor_tensor(
                    out=xo[:].rearrange("p (b f) -> p b f", b=NB),
                    in0=ft[:].rearrange("p (b f) -> p b f", b=NB),
                    in1=dvt["pdv0"][:, t0:t0 + NB].to_broadcast([P, NB, IN]),
                    op=OP.mult)
                nc.sync.dma_start(
                    xd0_sh[:].rearrange("b (two f) -> (b two) f", two=2)
                    .rearrange("(b p) f -> p b f", p=P)[:, t0:t0 + NB, :],
                    xo[:].rearrange("p (b f) -> p b f", b=NB))
                nc.scalar.activation(x0_sb[:, t0 * IN:(t0 + NB) * IN], ft[:],
                                     AF.Copy)
                if SPLITAG:
                    for h in range(_AGK - 1):
                        if t0 + NB == (h + 1) * T0 // _AGK:
                            ag_part(xd0_sh, xd0_full, PRSH, h)
            if not SPLITAG:
                for h in range(_AGK - 1):
                    ag_part(xd0_sh, xd0_full, PRSH, h)
            ag_part(xd0_sh, xd0_full, PRSH, _AGK - 1)

            # ================= layer 0, prop 1 =================
            def fin0_p1(t, seg):
                nc.vector.tensor_tensor(
                    out=x1_sb[:, t * IN:(t + 1) * IN], in0=seg[:],
                    in1=dvt["ndv0"][:, t:t + 1].to_broadcast([P, IN]), op=OP.mult)
                x1d = sb.tile([P, IN], BF16, tag="fin", name="f0b")
                nc.vector.tensor_tensor(
                    out=x1d[:], in0=seg[:],
                    in1=dvt["nd2v0"][:, t:t + 1].to_broadcast([P, IN]), op=OP.mult)
                nc.sync.dma_start(
                    x1d_sh[:].rearrange("b (two f) -> (b two) f", two=2)
                    [t * P:(t + 1) * P, :], x1d[:])
                if SPLITAG:
                    for h in range(_AGK - 1):
                        if t == (h + 1) * T0 // _AGK - 1:
                            ag_part(x1d_sh, x1d_full, PRSH, h)

            _emit_prop(nc, sb, ps, pl0, (xd0_full, idx0_t, dl0_t), gstate,
                       IN, fin0_p1)
            if not SPLITAG:
                for h in range(_AGK - 1):
                    ag_part(x1d_sh, x1d_full, PRSH, h)
            ag_part(x1d_sh, x1d_full, PRSH, _AGK - 1)

            # ====== layer 0, prop 2 (dense + pooling fused in) ======
            def dense0(t):
                xf = sb.tile([P, 3 * IN], F32, tag="lxf", name="lxf")
                for i, src in enumerate([x0_sb, x1_sb, x2_sb]):
                    nc.scalar.activation(xf[:, i * IN:(i + 1) * IN],
                                         src[:, t * IN:(t + 1) * IN], AF.Copy)
                trA = ps.tile([P, P], F32, tag="trA")
                nc.tensor.transpose(out=trA[: 2 * IN, :], in_=xf[:, : 2 * IN],
                                    identity=ident[:])
                xcatA = sb.tile([P, P], F32, tag="xcatA", name="xcatA")
                nc.scalar.activation(xcatA[: 2 * IN, :], trA[: 2 * IN, :], AF.Copy)
                trB = ps.tile([P, P], F32, tag="trA")
                nc.tensor.transpose(out=trB[: IN, :], in_=xf[:, 2 * IN:],
                                    identity=ident[:])
                xcatB = sb.tile([P, P], F32, tag="xcatB", name="xcatB")
                nc.scalar.activation(xcatB[: IN, :], trB[: IN, :], AF.Copy)
                hps = ps.tile([P, H], F32, tag="hps")
                nc.tensor.matmul(out=hps[:], lhsT=xcatA[: 2 * IN, :],
                                 rhs=w0a[: 2 * IN, :], start=True, stop=False)
                nc.tensor.matmul(out=hps[:], lhsT=xcatB[: IN, :],
                                 rhs=w0b[: IN, :], start=False, stop=True)
                hs0 = sb.tile([P, H], F32, tag="hsb0", name="hsb0")
                nc.vector.tensor_tensor(out=hs0[:], in0=hps[:], in1=b0_sb[:],
                                        op=OP.add)
                hsb = sb.tile([P, H], BF16, tag="hsb", name="hsb")
                nc.scalar.activation(hsb[:], hs0[:], AF.Relu)
                nc.sync.dma_start(hbuf[t * P:(t + 1) * P, :], hsb[:])

            def pool0(tp):
                ev = sb.tile([P, H], BF16, tag="pev", name="pev")
                nc.sync.dma_start(
                    ev[:], hbuf[:].rearrange("(n two) h -> n two h", two=2)
                    [tp * P:(tp + 1) * P, 0, :])
                od = sb.tile([P, H], BF16, tag="pod", name="pod")
                nc.sync.dma_start(
                    od[:], hbuf[:].rearrange("(n two) h -> n two h", two=2)
                    [tp * P:(tp + 1) * P, 1, :])
                nc.vector.tensor_tensor(out=xp_sb[:, tp * H:(tp + 1) * H],
                                        in0=ev[:], in1=od[:], op=OP.max)
                xpd = sb.tile([P, H], BF16, tag="pxd", name="pxd")
                nc.vector.tensor_tensor(
                    out=xpd[:], in0=xp_sb[:, tp * H:(tp + 1) * H],
                    in1=dvt["pdv1s"][:, tp:tp + 1].to_broadcast([P, H]),
                    op=OP.mult)
                nc.sync.dma_start(xpd_sh[tp * P:(tp + 1) * P, :], xpd[:])

            def fin0_p2(t, seg):
                x2t = sb.tile([P, IN], BF16, tag="fin", name="f0c")
                nc.vector.tensor_tensor(
                    out=x2t[:], in0=seg[:],
                    in1=dvt["n2dv0"][:, t:t + 1].to_broadcast([P, IN]), op=OP.mult)
                nc.vector.tensor_tensor(
                    out=x2_sb[:, t * IN:(t + 1) * IN], in0=x2t[:],
                    in1=x0_sb[:, t * IN:(t + 1) * IN], op=OP.subtract)
                dense0(t)
                if t % 2 == 1:
                    pool0(t // 2)
                    if SPLITAG:
                        for h in range(_AGK - 1):
                            if t == (h + 1) * T0 // _AGK - 1:
                                ag_part(xpd_sh, xpd_full, SH1, h)

            _emit_prop(nc, sb, ps, pl0, (x1d_full, idx0_t, dl0_t), gstate,
                       IN, fin0_p2)
            if not SPLITAG:
                for h in range(_AGK - 1):
                    ag_part(xpd_sh, xpd_full, SH1, h)
            ag_part(xpd_sh, xpd_full, SH1, _AGK - 1)

            # ================= layer 1, prop 1 =================
            def fin1_p1(t, seg):
                nc.vector.tensor_tensor(
                    out=x11_sb[:, t * H:(t + 1) * H], in0=seg[:],
                    in1=dvt["ndv1"][:, t:t + 1].to_broadcast([P, H]), op=OP.mult)
                x1d = sb.tile([P, H], BF16, tag="fin", name="f1b")
                nc.vector.tensor_tensor(
                    out=x1d[:], in0=seg[:],
                    in1=dvt["nd2v1"][:, t:t + 1].to_broadcast([P, H]), op=OP.mult)
                nc.sync.dma_start(x1d1_sh[t * P:(t + 1) * P, :], x1d[:])
                if SPLITAG:
                    for h in range(_AGK - 1):
                        if t == (h + 1) * T1 // _AGK - 1:
                            ag_part(x1d1_sh, x1d1_full, SH1, h)

            _emit_prop(nc, sb, ps, pl1, (xpd_full, idx1_t, dl1_t), gstate,
                       H, fin1_p1)
            if not SPLITAG:
                for h in range(_AGK - 1):
                    ag_part(x1d1_sh, x1d1_full, SH1, h)
            ag_part(x1d1_sh, x1d1_full, SH1, _AGK - 1)

            # ====== layer 1, prop 2 (dense + gmax fused in) ======
            gmaxn = sb1.tile([P, H], F32, name="gmaxn")
            nc.vector.memset(gmaxn[:], -3.0e38)

            def dense1(t):
                hps = ps.tile([P, H], F32, tag="hps")
                for i, src in enumerate([xp_sb, x11_sb, x21_sb]):
                    xf = sb.tile([P, H], F32, tag="lxf", name=f"l1f{i}")
                    nc.scalar.activation(xf[:], src[:, t * H:(t + 1) * H],
                                         AF.Copy)
                    tr = ps.tile([P, P], F32, tag="trA")
                    nc.tensor.transpose(out=tr[:], in_=xf[:], identity=ident[:])
                    xT = sb.tile([P, P], F32, tag="xcatA", name=f"m1T{i}")
                    nc.scalar.activation(xT[:], tr[:], AF.Copy)
                    nc.tensor.matmul(out=hps[:], lhsT=xT[:],
                                     rhs=w1_sb[:, i * H:(i + 1) * H],
                                     start=(i == 0), stop=(i == 2))
                hs0 = sb.tile([P, H], F32, tag="hsb0", name="m1h0")
                nc.vector.tensor_tensor(out=hs0[:], in0=hps[:], in1=b1_sb[:],
                                        op=OP.add)
                hsb = sb.tile([P, H], F32, tag="hsb2", name="m1h")
                nc.scalar.activation(hsb[:], hs0[:], AF.Relu)
                nc.vector.tensor_tensor(out=gmaxn[:], in0=gmaxn[:], in1=hsb[:],
                                        op=OP.max)

            def fin1_p2(t, seg):
                x2t = sb.tile([P, H], BF16, tag="fin", name="f1c")
                nc.vector.tensor_tensor(
                    out=x2t[:], in0=seg[:],
                    in1=dvt["n2dv1"][:, t:t + 1].to_broadcast([P, H]), op=OP.mult)
                nc.vector.tensor_tensor(
                    out=x21_sb[:, t * H:(t + 1) * H], in0=x2t[:],
                    in1=xp_sb[:, t * H:(t + 1) * H], op=OP.subtract)
                dense1(t)

            _emit_prop(nc, sb, ps, pl1, (x1d1_full, idx1_t, dl1_t), gstate,
                       H, fin1_p2)

            trg = ps.tile([P, P], F32, tag="trA")
            nc.tensor.transpose(out=trg[:], in_=gmaxn[:], identity=ident[:])
            gmax = sb1.tile([P, 1], F32, name="gmax")
            nc.vector.tensor_reduce(out=gmax[:], in_=trg[:], axis=AX.X, op=OP.max)
            nc.sync.dma_start(gmax_in[:], gmax[:])
            nc.gpsimd.collective_compute(
                "AllReduce", OP.max, replica_groups=groups,
                ins=[gmax_in[:].opt()], outs=[gmax_out[:].opt()])
            gmax2 = sb1.tile([P, 1], F32, name="gmax2")
            nc.sync.dma_start(gmax2[:], gmax_out[:])

            zps = ps.tile([1, OUT], F32, tag="seg")
            nc.tensor.matmul(out=zps[:], lhsT=gmax2[:], rhs=wc_sb[:, :OUT],
                             start=True, stop=True)
            z = sb1.tile([1, OUT], F32, name="zrow")
            nc.vector.tensor_tensor(out=z[:], in0=zps[:], in1=bc_sb[:], op=OP.add)
            m = sb1.tile([1, 1], F32, name="mrow")
            nc.vector.tensor_reduce(out=m[:], in_=z[:], axis=AX.X, op=OP.max)
            zc = sb1.tile([1, OUT], F32, name="zcrow")
            nc.vector.tensor_tensor(out=zc[:], in0=z[:],
                                    in1=m[:].to_broadcast([1, OUT]),
                                    op=OP.subtract)
            ez = sb1.tile([1, OUT], F32, name="ezrow")
            nc.scalar.activation(ez[:], zc[:], AF.Exp)
            s = sb1.tile([1, 1], F32, name="srow")
            nc.vector.tensor_reduce(out=s[:], in_=ez[:], axis=AX.X, op=OP.add)
            ls = sb1.tile([1, 1], F32, name="lsrow")
            nc.scalar.activation(ls[:], s[:], AF.Ln)
            yv = sb1.tile([1, OUT], F32, name="yrow")
            nc.vector.tensor_tensor(out=yv[:], in0=zc[:],
                                    in1=ls[:].to_broadcast([1, OUT]),
                                    op=OP.subtract)
            nc.sync.dma_start(y_d[:], yv[:])

    nc.compile()
    return nc


# --------------------------------------------------------------------------
# host entry
# --------------------------------------------------------------------------

def prepare(feat, src0, dst0, src1, dst1, W0, b0, W1, b1, Wc, bc, NC=8):
    feat = np.asarray(feat, np.float32)
    src0 = np.asarray(src0)
    dst0 = np.asarray(dst0)
    src1 = np.asarray(src1)
    dst1 = np.asarray(dst1)
    N0, IN = feat.shape
    N1 = N0 // 2
    H = np.asarray(W0).shape[1]
    OUT = np.asarray(Wc).shape[1]
    SH0, SH1 = N0 // NC, N1 // NC
    T0, T1, TP = SH0 // P, SH1 // P, SH0 // 2 // P

    pl0, pc0 = _plan_layer(src0, dst0, N0, NC, pair=True,
                           rowN=N0 // 2, rowSH=SH0 // 2)
    pl1, pc1 = _plan_layer(src1, dst1, N1, NC, pair=False,
                           rowN=N1, rowSH=SH1)

    deg0 = np.bincount(dst0, minlength=N0).astype(np.float32)
    dinv0 = 1.0 / np.sqrt(np.maximum(deg0, 1.0))
    deg1 = np.bincount(dst1, minlength=N1).astype(np.float32)
    dinv1 = 1.0 / np.sqrt(np.maximum(deg1, 1.0))

    cfg = {"N0": N0, "N1": N1, "IN": IN, "H": H, "OUT": OUT, "NC": NC,
           "plan0": pl0, "plan1": pl1}
    nc = build_program(cfg)

    iota_np = np.broadcast_to(np.arange(P, dtype=np.float32),
                              (P, P)).astype(BF).copy()
    ident_np = np.eye(P, dtype=np.float32)

    in_maps = []
    for c in range(NC):
        m = {
            "feat_sh": feat[c * SH0:(c + 1) * SH0],
            "idx0": pc0[c]["idx"], "dl0": pc0[c]["dl"],
            "idx1": pc1[c]["idx"], "dl1": pc1[c]["dl"],
            "pdv0": _dinv_cols(dinv0, c * SH0, SH0),
            "ndv0": -_dinv_cols(dinv0, c * SH0, SH0),
            "nd2v0": -(_dinv_cols(dinv0, c * SH0, SH0) ** 2),
            "n2dv0": -2.0 * _dinv_cols(dinv0, c * SH0, SH0),
            "pdv1s": _dinv_cols(dinv1, c * SH1, SH1),
            "ndv1": -_dinv_cols(dinv1, c * SH1, SH1),
            "nd2v1": -(_dinv_cols(dinv1, c * SH1, SH1) ** 2),
            "n2dv1": -2.0 * _dinv_cols(dinv1, c * SH1, SH1),
            "W0": np.asarray(W0, np.float32),
            "b0r": np.broadcast_to(np.asarray(b0, np.float32), (P, H)).copy(),
            "W1": np.asarray(W1, np.float32),
            "b1r": np.broadcast_to(np.asarray(b1, np.float32), (P, H)).copy(),
            "Wc": np.asarray(Wc, np.float32),
            "bcr": np.asarray(bc, np.float32).reshape(1, OUT),
            "iotaB": iota_np,
            "ident": ident_np,
        }
        in_maps.append(m)
    return nc, in_maps


def run(feat, src0, dst0, src1, dst1, W0, b0, W1, b1, Wc, bc, NC=8, **rkw):
    nc, in_maps = prepare(feat, src0, dst0, src1, dst1, W0, b0, W1, b1,
                          Wc, bc, NC)
    res = bass_utils.run_bass_kernel_spmd(nc, in_maps,
                                          core_ids=list(range(NC)), **rkw)
    return res.results[0]["y"], res


def kernel(**inputs):
    y, _ = run(**inputs)
    return y

